# revision 28
# baseline (speedup 1.0000x reference)
"""Bass/Trainium2 kernel for nn_DiagonalTraining (per-anti-diagonal Linear).

Math: for each anti-diagonal i of x[B,S,S] (entries x[b,r,i-r], r<=i),
apply Linear_i (weights W[i,:i+1,:i+1], bias b[i,:i+1]) to the gathered
vector and scatter back reversed. Equivalent to:
    D[b,i,j] = x[b,j,i-j] (j<=i else 0)
    out[b,i,k] = sum_j W[i,k,j] * D[b,i,j] + b[i,k]
    new_x[b,r,c] = out[b,r+c,c] if r+c < S else x[b,r,c]

Device does the einsum (memory-bound: streams the valid triangle of W);
gather/scatter/bias are tiny O(S^2) host ops.

Sharding: interleaved over diagonals — core c owns i = c, c+8, ..., c+504
(slot m holds diagonal 8m+c, k-padded to L=8(m+1)). All cores run one
identical SPMD program; padding rows/cols of W and D are zero by
construction so results are exact.

Performance architecture (measured on HW):
- Everything fp8e4: W scaled by 32 on host (scale-invariant quantization,
  rel err 9.2e-3 from W alone), D bf16, and the PSUM result out*32 fits
  fp8e4 output staging directly (total rel err 1.30e-2 < 2e-2).
- The PE with 4-way tile_position col-packing streams ~2.5 cols/cycle
  (~770GB/s of fp8 W) — 1.8x the ~420GB/s DMA rate, so the kernel is
  stream-paced end to end. Only stream continuity, startup latency and
  the post-last-byte tail matter.
- W image is packed per-superchunk as [full-height chunk block][partial
  last-chunk block at the group-shared height H]: PE streaming time is
  per COLUMN (independent of partition height), so trimming the mostly
  -zero last j-chunks saves ~13% of W traffic for free. Matmuls slice
  both operands to the transferred height (never read unwritten SBUF).
- Each superchunk is fetched as big DMAs split across the sync+scalar
  queues (multi-KB per-partition descriptors, both queues saturated).
  Small groups stream first (instant PE start from resident wsmall),
  the tiny g4 is streamed last so the post-stream tail is short.
- Mid-loop output DMAs go on gpsimd only (a compute-gated descriptor on
  a W queue would stall all later W transfers behind it); the last two
  processed groups' outputs use the by-then-idle sync/scalar queues.
"""

import sys

sys.path.insert(0, "/opt/trn_rl_repo")

import numpy as np

B = 8
S = 512
NCORES = 8
M = 64  # diagonal slots per core
LBAR = [8 * (m + 1) for m in range(M)]  # k-padded diagonal length per slot
NQ = [1 if m < 16 else (m // 16 + 1) for m in range(M)]  # j-chunks per slot
QOFF = np.cumsum([0] + NQ).tolist()  # chunk index offset per slot in dt image
DTOT = QOFF[M]  # 160 chunks
# wsmall region (slots 0..15): column offsets within [0, SMALL_TOT)
WCUM = np.cumsum([0] + [NQ[m] * LBAR[m] for m in range(M)]).tolist()
SMALL_TOT = WCUM[16]  # 1088
G = 16  # groups of 4 slots sharing a PSUM bank
LG = [32 * (g + 1) for g in range(G)]  # group output width
OCUM = np.cumsum([0] + LG).tolist()
OTOT = OCUM[G]  # 4352
WSCALE = 32.0  # fp8 W scale; PSUM holds out*32 which fits fp8e4 directly

# Stream order of the W superchunks (all groups in a chunk share nq).
# Small resident groups 0-3 are computed first; g4 streams LAST so the
# post-stream tail (PE+copy+out of the final group) is minimal.
SUPER = [[5, 6, 7], [8, 9], [10, 11], [12], [13], [14], [15], [4]]
PROC_ORDER = [3, 2, 1, 0] + [g for ch in SUPER for g in ch]


def _build_wlayout():
    """Column layout of the m>=16 W image: per superchunk, the full-height
    chunks of all its slots, then the partial last chunks laid out per
    group (transferred at each group's own height H_g = 32*(g+1) -
    128*(nq-1); rows beyond a slot's true j-extent are zero)."""
    scol = {}
    info = {}
    col = SMALL_TOT
    for ch in SUPER:
        nq = NQ[4 * ch[0]]
        f0 = col
        for g in ch:
            for t in range(4):
                m = 4 * g + t
                for q in range(nq - 1):
                    scol[(m, q)] = col
                    col += LBAR[m]
        parts = {}
        for g in ch:
            gp0 = col
            for t in range(4):
                m = 4 * g + t
                scol[(m, nq - 1)] = col
                col += LBAR[m]
            parts[g] = (gp0, col, 32 * (g + 1) - 128 * (nq - 1))
        info[tuple(ch)] = (f0, col, parts)
    return scol, info, col


SCOL, CHINFO, WTOT2 = _build_wlayout()
HPART = {g: p[2] for ch in SUPER for g, p in CHINFO[tuple(ch)][2].items()}
HSMALL = [32 * (g + 1) for g in range(4)]  # wsmall transfer heights

MODE = "fp8"
_compiled = {}


def build_program(mode=MODE):
    """Build the SPMD Bass program (same instructions on all 8 cores)."""
    import concourse.mybir as mybir
    import concourse.tile as tile
    from concourse import bacc

    assert mode == "fp8"
    wdt = mybir.dt.float8e4
    ddt = mybir.dt.bfloat16
    odt = mybir.dt.float8e4
    f32 = mybir.dt.float32

    nc = bacc.Bacc("TRN2")
    wimg = nc.dram_tensor("wimg", [128, WTOT2], wdt, kind="ExternalInput")
    dt_in = nc.dram_tensor("dt", [128, DTOT * B], ddt, kind="ExternalInput")
    out = nc.dram_tensor("out", [128, OTOT], odt, kind="ExternalOutput")

    with tile.TileContext(nc) as tc:
        with (
            tc.tile_pool(name="dpool", bufs=1) as dpool,
            tc.tile_pool(name="wspool", bufs=1) as wspool,
            tc.tile_pool(name="wpool", bufs=8) as wpool,
            tc.tile_pool(name="opool", bufs=16) as opool,
            tc.tile_pool(name="psum", bufs=8, space="PSUM") as psum_pool,
        ):
            # Tiny gating inputs first: the m<16 slice of D, then wsmall,
            # then the rest of D — split across both queues.
            dtall = dpool.tile([128, DTOT * B], ddt)
            dsplit = QOFF[16] * B
            dmid = dsplit + (DTOT * B - dsplit) // 2
            nc.sync.dma_start(dtall[:, 0:dsplit], dt_in[:, 0:dsplit])
            wsmall = wspool.tile([128, SMALL_TOT], wdt)
            for g in range(4):
                eng = nc.scalar if g % 2 == 0 else nc.sync
                c0, c1 = WCUM[4 * g], WCUM[4 * g + 4]
                eng.dma_start(
                    wsmall[0 : HSMALL[g], c0:c1], wimg[0 : HSMALL[g], c0:c1]
                )
            nc.sync.dma_start(dtall[:, dsplit:dmid], dt_in[:, dsplit:dmid])
            nc.scalar.dma_start(dtall[:, dmid:], dt_in[:, dmid:])

            n_pdma = 0

            def fetch_chunk(ch):
                """Fetch one superchunk: full-height block split across both
                queues, then each group's partial block at its own height."""
                nonlocal n_pdma
                f0, c1, parts = CHINFO[tuple(ch)]
                p0 = min(gp0 for gp0, _, _ in parts.values())
                wtile = wpool.tile([128, 8544], wdt, tag="w")
                if p0 > f0:
                    fm = f0 + (p0 - f0) // 2
                    nc.sync.dma_start(wtile[0:128, 0 : fm - f0], wimg[:, f0:fm])
                    nc.scalar.dma_start(
                        wtile[0:128, fm - f0 : p0 - f0], wimg[:, fm:p0]
                    )
                for g in ch:
                    gp0, gp1, H = parts[g]
                    eng = nc.sync if n_pdma % 2 == 0 else nc.scalar
                    n_pdma += 1
                    eng.dma_start(
                        wtile[0:H, gp0 - f0 : gp1 - f0], wimg[0:H, gp0:gp1]
                    )
                return wtile

            CHUNK_OF = {g: tuple(ch) for ch in SUPER for g in ch}
            fetched = {}
            for g in PROC_ORDER:
                ps = psum_pool.tile([128, 512], f32, tag="ps")
                if g >= 4:
                    ch = CHUNK_OF[g]
                    if ch not in fetched:
                        fetched[ch] = fetch_chunk(ch)
                    wtile = fetched[ch]
                    f0 = CHINFO[ch][0]
                    H = HPART[g]
                for t in range(4):
                    m = 4 * g + t
                    L = LBAR[m]
                    nq = NQ[m]
                    for q in range(nq):
                        if m < 16:
                            h = HSMALL[g]
                            rhs = wsmall[0:h, WCUM[m] : WCUM[m] + L]
                        else:
                            h = H if q == nq - 1 else 128
                            c0 = SCOL[(m, q)] - f0
                            rhs = wtile[0:h, c0 : c0 + L]
                        nc.tensor.matmul(
                            ps[32 * t : 32 * t + B, 0:L],
                            lhsT=dtall[
                                0:h, (QOFF[m] + q) * B : (QOFF[m] + q + 1) * B
                            ],
                            rhs=rhs,
                            start=(q == 0),
                            stop=(q == nq - 1),
                            tile_position=(0, 32 * t),
                        )
                ot = opool.tile([128, 512], odt, tag="ostage")
                if g % 2 == 1:
                    nc.vector.tensor_copy(ot[0:128, 0 : LG[g]], ps[0:128, 0 : LG[g]])
                else:
                    nc.scalar.copy(ot[0:128, 0 : LG[g]], ps[0:128, 0 : LG[g]])
                # Mid-loop out DMAs must not share the W queues (in-order
                # queues: a compute-gated descriptor would stall later W
                # transfers). The last two processed groups are emitted
                # after all W fetches, so sync/scalar are safe and faster
                # than the tail of gpsimd's issue chain.
                oeng = {15: nc.sync, 4: nc.scalar}.get(g, nc.gpsimd)
                oeng.dma_start(out[:, OCUM[g] : OCUM[g + 1]], ot[0:128, 0 : LG[g]])

    nc.compile()
    return nc


def _get_program(mode=MODE):
    if mode not in _compiled:
        _compiled[mode] = build_program(mode)
    return _compiled[mode]


def _prep_inputs(x, W, mode=MODE):
    """Host-side shard prep: gather diagonals of x, pack W SBUF images."""
    import ml_dtypes

    wnp = np.dtype(ml_dtypes.float8_e4m3)
    dnp = np.dtype(ml_dtypes.bfloat16)
    wscale = np.float32(WSCALE)

    i_idx = np.arange(S)[:, None]
    r_idx = np.arange(S)[None, :]
    cols = (i_idx - r_idx) % S
    valid = (r_idx <= i_idx)[None]
    D = np.where(valid, x[:, r_idx, cols], np.float32(0.0))  # [B, S(i), S(j)]

    in_maps = []
    for c in range(NCORES):
        Wc = W[c::8]  # [M, S(k), S(j)]
        WIMG = np.zeros((128, WTOT2), dtype=wnp)
        for m in range(M):
            L, nq = LBAR[m], NQ[m]
            for q in range(nq):
                # img[j, k] = Wc[m, k, 128q + j] * wscale
                blk = Wc[m, 0:L, 128 * q : 128 * (q + 1)] * wscale  # [k, j]
                img = blk.T.astype(wnp, copy=False)  # [j<=128, k=L]
                jh = img.shape[0]
                c0 = WCUM[m] + q * L if m < 16 else SCOL[(m, q)]
                WIMG[0:jh, c0 : c0 + L] = img
        # DT[j, qoff_m + q, b] = D[b, 8m+c, 128q+j], used chunks only
        Dc = D[:, c::8, :]  # [B, M, S]
        DT = np.empty((128, DTOT * B), dtype=dnp)
        for m in range(M):
            nq = NQ[m]
            blk = Dc[:, m, 0 : 128 * nq]  # [B, 128*nq]
            arr = blk.T.reshape(nq, 128, B).transpose(1, 0, 2).reshape(128, nq * B)
            DT[:, QOFF[m] * B : (QOFF[m] + nq) * B] = arr.astype(dnp, copy=False)
        in_maps.append({"wimg": WIMG, "dt": DT})
    return in_maps


def _postprocess(x, bvec, results, mode=MODE):
    """Assemble per-core outputs, undo W scale, add bias, scatter back."""
    inv_scale = np.float32(1.0 / WSCALE)
    out_full = np.empty((B, S, S), dtype=np.float32)
    for c in range(NCORES):
        o = np.asarray(results[c]["out"]).astype(np.float32)  # [128, OTOT]
        for g in range(G):
            blk = o[:, OCUM[g] : OCUM[g + 1]].reshape(4, 32, LG[g])[:, 0:B]
            for t in range(4):
                m = 4 * g + t
                out_full[:, 8 * m + c, 0 : LBAR[m]] = blk[t, :, 0 : LBAR[m]]
    out_full *= inv_scale
    out_full += bvec[None]
    rr = np.arange(S)[:, None]
    cc = np.arange(S)[None, :]
    diag = rr + cc
    new_x = np.where(
        (diag < S)[None], out_full[:, np.minimum(diag, S - 1), cc], x
    ).astype(np.float32)
    return new_x


def kernel_run(x, W, b, mode=MODE, trace=False):
    from concourse.bass_utils import run_bass_kernel_spmd

    nc = _get_program(mode)
    in_maps = _prep_inputs(x, W, mode)
    res = run_bass_kernel_spmd(nc, in_maps, list(range(NCORES)), trace=trace)
    return _postprocess(x, b, res.results, mode), res


def kernel(x, W, b):
    out, _ = kernel_run(np.asarray(x), np.asarray(W), np.asarray(b))
    return out
